# revision 11
# baseline (speedup 1.0000x reference)
"""Trainium2 Bass kernel for nn_AudioModelM3 (2-layer Mamba2 + heads).

Sharding: batch 8 -> 8 NeuronCores (data parallel, replicated weights).
Per core: one [1024(seq), 1024(d_model)] sequence through two Mamba2 blocks,
selu, mean-pool, and the two linear heads -> [10] logits.

Layouts: projections + depthwise conv run feature-major [feature, time];
the SSD chunk math runs time-major per (head, chunk) via one PE transpose of
xs; per-head scalar vectors (dA, decay, exp(Acs), dt) are packed into a
[128, 1024] tile and transposed once. The inter-chunk state scan uses the
DVE tensor_tensor_scan with zeroed decay at segment starts. z and y round-
trip through DRAM to stay inside SBUF.
"""
import numpy as np

import concourse.bacc as bacc
import concourse.bass as bass
import concourse.mybir as mybir
import concourse.tile as tile
from concourse import bass_utils

f32 = mybir.dt.float32
bf16 = mybir.dt.bfloat16
AL = mybir.AluOpType
AF = mybir.ActivationFunctionType
AX = mybir.AxisListType

L, DM, DI, DS, NH, HD = 1024, 1024, 2048, 128, 32, 64
NCH, CH = 16, 64
DIP = 2 * DI + 2 * DS + NH          # 4384
SELU_S = 1.0507009873554805
SELU_A = 1.6732632423543772
SA = SELU_S * SELU_A
HG = 2                               # heads per scan group
NHG = NH // HG

_CACHE = {}


def _emit_layer(nc, P, dram, li, xf_tiles):
    pool, p1, psA, psB, psC = P["pool"], P["p1"], P["psA"], P["psB"], P["psC"]
    big8, xfp = P["big8"], P["xfp"]
    id_bf, id_f32, q2, pt_bf, segmask = (P["id_bf"], P["id_f32"], P["q2"],
                                         P["pt_bf"], P["segmask"])
    z_dram = P["dramp"].tile([L, DI], bf16, tag=f"zdram{li}", name=f"zdram{li}")[:]
    y_dram = P["dramp"].tile([L, DI], bf16, tag=f"ydram{li}", name=f"ydram{li}")[:]

    convw = p1.tile([128, 72], f32, tag="convw", name="convw")
    convb = p1.tile([128, 36], f32, tag="convb", name="convb")
    dtb = p1.tile([32, 1], f32, tag="dtb", name="dtb")
    Acoef = p1.tile([32, 1], f32, tag="Acoef", name="Acoef")
    drep = p1.tile([128, 32], f32, tag="drep", name="drep")
    nc.sync.dma_start(convw[:], dram[f"l{li}_convw"].ap())
    nc.sync.dma_start(convb[:], dram[f"l{li}_convb"].ap())
    nc.sync.dma_start(dtb[:], dram[f"l{li}_dtb"].ap())
    nc.sync.dma_start(Acoef[:], dram[f"l{li}_A"].ap())
    nc.sync.dma_start(drep[:], dram[f"l{li}_Drep"].ap())
    inw_dram = dram[f"l{li}_inwT"].ap()
    outw_dram = dram[f"l{li}_outwT"].ap()

    xtm = [big8.tile([128, 2048], bf16, tag="big8", name="big8") for _ in range(8)]
    bt = p1.tile([128, 1024], bf16, tag="bt", name="bt")
    ct = p1.tile([128, 1024], bf16, tag="ct", name="ct")
    btm2 = p1.tile([128, 16, 128], bf16, tag="btm2", name="btm2")
    nc.vector.memset(btm2[:], 0.0)
    dtraw = pool.tile([32, 1024], f32, tag="sh_dtraw_zt", name="sh_dtraw_zt")

    # ============ Phase A: in_proj (+ conv + xs/B transposes) ============
    for f0, fw in [(k * 512, 512) for k in range(8)] + [(4096, 288)]:
        inw = [P["inwp"].tile([128, 512], bf16, tag="inw", name="inw") for _ in range(8)]
        for dk in range(8):
            nc.sync.dma_start(inw[dk][:, :fw],
                              inw_dram[dk * 128:(dk + 1) * 128, f0:f0 + fw])
        if f0 < 2048:                      # --- z part (time-major) ---
            for lt in range(8):
                pz = psA.tile([128, 512], f32, tag="mm", name="mm")
                for dk in range(8):
                    nc.tensor.matmul(pz[:], xf_tiles[dk][:, lt * 128:(lt + 1) * 128],
                                     inw[dk][:, :512], start=(dk == 0), stop=(dk == 7))
                zsb = pool.tile([128, 512], bf16, tag="zsb", name="zsb")
                nc.scalar.copy(zsb[:], pz[:])
                nc.sync.dma_start(z_dram[lt * 128:(lt + 1) * 128, f0:f0 + 512], zsb[:])
        else:                              # --- conv-channel / dt parts ---
            if f0 < 4096:
                fts = [(f0 - 2048) // 128 + j for j in range(4)]
                locs = [j * 128 for j in range(4)]
            else:
                fts, locs = [16, 17], [0, 128]
            for ft, loc in zip(fts, locs):
                pad = pool.tile([128, 1027], bf16, tag="pad", name="pad")
                nc.vector.memset(pad[:, 0:3], 0.0)
                for ng in range(2):
                    px = psA.tile([128, 512], f32, tag="mm", name="mm")
                    for dk in range(8):
                        nc.tensor.matmul(px[:], inw[dk][:, loc:loc + 128],
                                         xf_tiles[dk][:, ng * 512:(ng + 1) * 512],
                                         start=(dk == 0), stop=(dk == 7))
                    nc.scalar.copy(pad[:, 3 + ng * 512:3 + (ng + 1) * 512], px[:])
                # causal depthwise conv + bias + silu
                acc = pool.tile([128, 1024], f32, tag="sh_cacc_selu", name="sh_cacc_selu")
                nc.vector.tensor_scalar_mul(acc[:], pad[:, 0:1024],
                                            convw[:, ft * 4:ft * 4 + 1])
                for k in (1, 2, 3):
                    nc.vector.scalar_tensor_tensor(
                        acc[:], pad[:, k:k + 1024],
                        convw[:, ft * 4 + k:ft * 4 + k + 1], acc[:], AL.mult, AL.add)
                if ft < 16:
                    dest = pool.tile([128, 1024], bf16, tag="xsf", name="xsf")
                elif ft == 16:
                    dest = bt
                else:
                    dest = ct
                for q in range(2):
                    sl = slice(q * 512, (q + 1) * 512)
                    u = pool.tile([128, 512], bf16, tag="cu", name="cu")
                    e = pool.tile([128, 512], f32, tag="ce", name="ce")
                    nc.scalar.activation(u[:], acc[:, sl], AF.Identity,
                                         bias=convb[:, ft:ft + 1], scale=1.0)
                    nc.scalar.activation(e[:], acc[:, sl], AF.Exp,
                                         bias=convb[:, 18 + ft:19 + ft], scale=-1.0)
                    nc.vector.tensor_scalar_add(e[:], e[:], 1.0)
                    r = pool.tile([128, 512], f32, tag="ce", name="ce")
                    nc.vector.reciprocal(r[:], e[:])
                    nc.vector.tensor_tensor(dest[:, sl], u[:], r[:], AL.mult)
                if ft < 16:
                    for p in range(8):
                        ptr = psC.tile([128, 128], bf16, tag="trb", name="trb")
                        nc.tensor.transpose(ptr[:], dest[:, p * 128:(p + 1) * 128],
                                            id_bf[:])
                        nc.scalar.copy(xtm[p][:, ft * 128:(ft + 1) * 128], ptr[:])
                elif ft == 16:
                    for c in range(NCH):
                        ptr = psC.tile([128, 128], bf16, tag="trb", name="trb")
                        half = (c % 2) * 64
                        nc.tensor.transpose(ptr[half:half + 64, :],
                                            bt[:, c * 64:(c + 1) * 64], id_bf[:])
                        nc.scalar.copy(btm2[half:half + 64, c, :],
                                       ptr[half:half + 64, :])
            if f0 >= 4096:                 # --- dt part ---
                for ng in range(2):
                    pd = psA.tile([128, 512], f32, tag="mm", name="mm")
                    for dk in range(8):
                        nc.tensor.matmul(pd[:32, :], inw[dk][:, 256:288],
                                         xf_tiles[dk][:, ng * 512:(ng + 1) * 512],
                                         start=(dk == 0), stop=(dk == 7))
                    nc.scalar.copy(dtraw[:, ng * 512:(ng + 1) * 512], pd[:32, :])

    # ============ Phase C: dt pipeline -> packed time-major scalars ======
    tmp1 = p1.tile([32, 1024], f32, tag="dttmp1", name="dttmp1")
    dt_t = p1.tile([32, 1024], f32, tag="dt_t", name="dt_t")
    dA_t = p1.tile([32, 1024], f32, tag="dA_t", name="dA_t")
    dfac_t = p1.tile([32, 1024], f32, tag="dfac_t", name="dfac_t")
    eacs_t = p1.tile([32, 1024], f32, tag="eacs_t", name="eacs_t")
    nc.scalar.activation(tmp1[:], dtraw[:], AF.Exp, bias=dtb[:], scale=1.0)
    nc.scalar.activation(dt_t[:], tmp1[:], AF.Ln, bias=1.0, scale=1.0)
    nc.vector.tensor_scalar_mul(dA_t[:], dt_t[:], Acoef[:])
    acs = pool.tile([32, 1024], f32, tag="sh_acs_outw", name="sh_acs_outw")
    nc.vector.tensor_tensor_scan(acs[:], segmask[:], dA_t[:], 0.0,
                                 AL.mult, AL.add)
    lastb = acs[:, 63::64].unsqueeze(2).broadcast_to([32, 16, 64])
    d0 = p1.tile([32, 1024], f32, tag="dttmp2", name="dttmp2")
    nc.vector.tensor_tensor(d0[:].rearrange("p (c q) -> p c q", c=16), lastb,
                            acs[:].rearrange("p (c q) -> p c q", c=16), AL.subtract)
    nc.scalar.activation(tmp1[:], d0[:], AF.Exp)
    nc.vector.tensor_tensor(dfac_t[:], tmp1[:], dt_t[:], AL.mult)
    nc.scalar.activation(eacs_t[:], acs[:], AF.Exp)
    # transpose the four [32,1024] quantity maps to time-major [128, 8, 32]
    tmq = []
    for qi, qt in enumerate((dA_t, dfac_t, eacs_t, dt_t)):
        tq = p1.tile([128, 8, 32], f32, tag=f"tmq{qi}", name=f"tmq{qi}")
        for i in range(8):
            ptr = psC.tile([128, 128], f32, tag="trd", name="trd")
            nc.tensor.transpose(ptr[:, 0:32], qt[:, i * 128:(i + 1) * 128],
                                id_f32[:32, :32])
            nc.scalar.copy(tq[:, i, :], ptr[:, 0:32])
        tmq.append(tq)
    tm_dA, tm_dfac, tm_eacs, tm_dt = tmq
    decrow = p1.tile([1, 512], f32, tag="decrow", name="decrow")
    nc.sync.dma_start(decrow[:].rearrange("o (h c) -> o h c", h=32),
                      eacs_t[:, 63::64])
    dec_rep = p1.tile([128, 512], f32, tag="dec_rep", name="dec_rep")
    nc.gpsimd.partition_broadcast(dec_rep[:], decrow[:])
    nc.vector.memset(dec_rep[:, 0:512:16], 0.0)

    # ====== GT: masked chunk grams, block-diag pair layout [128,(p,128)] ==
    gtmblk = p1.tile([128, 1024], f32, tag="gtmblk", name="gtmblk")
    pg = psB.tile([128, 1024], f32, tag="wide", name="wide")
    nc.vector.memset(pg[:], 0.0)
    for c in range(NCH):
        p, half = c // 2, (c % 2) * 64
        nc.tensor.matmul(pg[half:half + 64, p * 128 + half:p * 128 + half + 64],
                         bt[:, c * 64:(c + 1) * 64], ct[:, c * 64:(c + 1) * 64],
                         start=True, stop=True)
    ub = P["q2blk"][:].unsqueeze(1).broadcast_to([128, 8, 128])
    nc.vector.tensor_tensor(gtmblk[:].rearrange("p (c q) -> p c q", c=8),
                            pg[:].rearrange("p (c q) -> p c q", c=8), ub, AL.mult)

    # ============ Phase D: SSD per head-group ============================
    for hg in range(NHG):
        scan_in = p1.tile([128, HG * 1024], f32, tag="scan_in", name="scan_in")
        scan_out = p1.tile([128, HG * 1024], bf16, tag="scan_out", name="scan_out")
        mt_all = p1.tile([128, HG * 1024], bf16, tag="mt_all", name="mt_all")
        xcdt_all = p1.tile([128, HG * 512], bf16, tag="xcdt", name="xcdt")
        for hl in range(HG):
            h = hg * HG + hl
            daq = pool.tile([128, 512], bf16, tag="daq", name="daq")
            xcdec = pool.tile([128, 512], bf16, tag="xcdec", name="xcdec")
            for p in range(8):
                nc.vector.tensor_scalar_mul(daq[:, p * 64:(p + 1) * 64], q2[:],
                                            tm_dA[:, p, h:h + 1])
                nc.vector.tensor_scalar_mul(
                    xcdt_all[:, hl * 512 + p * 64:hl * 512 + (p + 1) * 64],
                    xtm[p][:, h * 64:(h + 1) * 64], tm_dt[:, p, h:h + 1])
                nc.vector.tensor_scalar_mul(xcdec[:, p * 64:(p + 1) * 64],
                                            xtm[p][:, h * 64:(h + 1) * 64],
                                            tm_dfac[:, p, h:h + 1])
            pdif = psB.tile([128, 1024], f32, tag="wide", name="wide")
            nc.vector.memset(pdif[:], 0.0)
            for c in range(NCH):
                p, half = c // 2, (c % 2) * 64
                nc.tensor.matmul(
                    pdif[half:half + 64, p * 128 + half:p * 128 + half + 64],
                    pt_bf[:, half:half + 64], daq[:, p * 64:(p + 1) * 64],
                    start=True, stop=True)
            ll = pool.tile([128, 1024], bf16, tag="ll", name="ll")
            nc.scalar.activation(ll[:], pdif[:], AF.Exp)
            nc.vector.tensor_tensor(mt_all[:, hl * 1024:(hl + 1) * 1024],
                                    ll[:], gtmblk[:], AL.mult)
            for grp in range(2):
                pst = psA.tile([128, 512], f32, tag="mm", name="mm")
                for j in range(8):
                    c = grp * 8 + j
                    nc.tensor.matmul(
                        pst[:, j * 64:(j + 1) * 64], btm2[:, c, :],
                        xcdec[:, (c // 2) * 64:(c // 2) * 64 + 64],
                        start=True, stop=True)
                out_ap = scan_in[:].rearrange("n (h d c) -> n h d c", h=HG, d=HD)[
                    :, hl, :, grp * 8:(grp + 1) * 8].transpose([0, 2, 1])
                nc.vector.tensor_copy(out_ap,
                                      pst[:].rearrange("n (c d) -> n c d", c=8))
            decb = p1.tile([128, 1024], f32, tag="decb", name="decb")
            nc.vector.tensor_copy(
                decb[:].rearrange("n (d c) -> n d c", d=HD),
                dec_rep[:, h * 16:(h + 1) * 16].unsqueeze(1)
                .broadcast_to([128, HD, 16]))
            nc.vector.tensor_tensor_scan(
                scan_out[:, hl * 1024:(hl + 1) * 1024], decb[:],
                scan_in[:, hl * 1024:(hl + 1) * 1024], 0.0, AL.mult, AL.add)

        # ---- Yo + Yd + D-skip combine, per chunk-pair ----
        for p in range(8):
            pyo = psA.tile([128, 512], f32, tag="mm", name="mm")
            pyd = psA.tile([128, 512], f32, tag="mm", name="mm")
            W = HG * HD                      # 128 cols
            for half in range(2):
                c = 2 * p + half
                osl = slice(half * 64, half * 64 + 64)
                if c == 0:
                    nc.vector.memset(pyo[osl, 0:W], 0.0)
                else:
                    rhs = scan_out[:].rearrange("n (h d c) -> n h d c",
                                                h=HG, d=HD)[:, :, :, c - 1]
                    nc.tensor.matmul(pyo[osl, 0:W], ct[:, c * 64:(c + 1) * 64],
                                     rhs, start=True, stop=True)
            for hl in range(HG):
                nc.tensor.matmul(
                    pyd[:, hl * 64:(hl + 1) * 64],
                    mt_all[:, hl * 1024 + p * 128:hl * 1024 + (p + 1) * 128],
                    xcdt_all[:, hl * 512 + p * 64:hl * 512 + (p + 1) * 64],
                    start=True, stop=True)
            eb = tm_eacs[:, p, hg * HG:(hg + 1) * HG].unsqueeze(2) \
                .broadcast_to([128, HG, HD])
            t1 = pool.tile([128, 128], f32, tag="comb1", name="comb1")
            nc.vector.tensor_tensor(t1[:].rearrange("n (h d) -> n h d", h=HG),
                                    pyo[:, 0:W].rearrange("n (h d) -> n h d", h=HG),
                                    eb, AL.mult)
            t2 = pool.tile([128, 128], f32, tag="comb2", name="comb2")
            nc.vector.tensor_tensor(t2[:], t1[:], pyd[:, 0:W], AL.add)
            db = drep[:, hg * HG:(hg + 1) * HG].unsqueeze(2) \
                .broadcast_to([128, HG, HD])
            t3 = pool.tile([128, 128], f32, tag="comb3", name="comb3")
            nc.vector.tensor_tensor(
                t3[:].rearrange("n (h d) -> n h d", h=HG),
                xtm[p][:, hg * W:(hg + 1) * W].rearrange("n (h d) -> n h d", h=HG),
                db, AL.mult)
            yout = pool.tile([128, 128], bf16, tag="yout", name="yout")
            nc.vector.tensor_tensor(yout[:], t2[:], t3[:], AL.add)
            nc.sync.dma_start(y_dram[p * 128:(p + 1) * 128, hg * W:(hg + 1) * W],
                              yout[:])

    # ============ Phase E: gating + rmsnorm + y transpose ================
    ynt = [big8.tile([128, 2048], bf16, tag="big8", name="big8") for _ in range(8)]
    for p in range(8):
        zt = pool.tile([128, 2048], bf16, tag="sh_dtraw_zt", name="sh_dtraw_zt")
        nc.sync.dma_start(zt[:], z_dram[p * 128:(p + 1) * 128, :])
        yin = pool.tile([128, 2048], bf16, tag="yin", name="yin")
        nc.sync.dma_start(yin[:], y_dram[p * 128:(p + 1) * 128, :])
        yg = pool.tile([128, 2048], bf16, tag="sh_pk_yg", name="sh_pk_yg")
        ss = pool.tile([128, 4], f32, tag="ss", name="ss")
        for q in range(4):
            sl = slice(q * 512, (q + 1) * 512)
            e = pool.tile([128, 512], bf16, tag="ge", name="ge")
            nc.scalar.activation(e[:], zt[:, sl], AF.Exp, scale=-1.0)
            nc.vector.tensor_scalar_add(e[:], e[:], 1.0)
            r = pool.tile([128, 512], f32, tag="gr", name="gr")
            nc.vector.reciprocal(r[:], e[:])
            sg = pool.tile([128, 512], bf16, tag="ge", name="ge")
            nc.vector.tensor_tensor(sg[:], zt[:, sl], r[:], AL.mult)
            nc.vector.tensor_tensor(yg[:, sl], yin[:, sl], sg[:], AL.mult)
            sqs = pool.tile([128, 512], f32, tag="gr", name="gr")
            nc.scalar.activation(sqs[:], yg[:, sl], AF.Square,
                                 accum_out=ss[:, q:q + 1])
        sstot = pool.tile([128, 2], f32, tag="sstot", name="sstot")
        nc.vector.tensor_reduce(sstot[:, 0:1], ss[:], AX.X, AL.add)
        nc.scalar.activation(sstot[:, 1:2], sstot[:, 0:1], AF.Ln,
                             bias=P["epsb"][:], scale=1.0 / DI)
        r2 = pool.tile([128, 1], f32, tag="r2", name="r2")
        nc.scalar.activation(r2[:], sstot[:, 1:2], AF.Exp, scale=-0.5)
        diag = pool.tile([128, 128], bf16, tag="diag", name="diag")
        nc.vector.tensor_scalar_mul(diag[:], id_bf[:], r2[:])
        for fk in range(16):
            ptr = psC.tile([128, 128], f32, tag="trd", name="trd")
            nc.tensor.matmul(ptr[:], yg[:, fk * 128:(fk + 1) * 128], diag[:],
                             start=True, stop=True)
            nc.scalar.copy(ynt[fk // 2][:, (fk % 2) * 1024 + p * 128:
                                        (fk % 2) * 1024 + (p + 1) * 128], ptr[:])

    # ============ Phase G: out_proj + selu -> new feature-major x ========
    new_xf = []
    for dk in range(8):
        ow = pool.tile([128, 16, 128], bf16, tag="sh_acs_outw", name="sh_acs_outw")
        nc.sync.dma_start(ow[:], outw_dram[:, dk * 128:(dk + 1) * 128]
                          .rearrange("(f p) d -> p f d", p=128))
        nxf = xfp.tile([128, 1024], bf16, tag="xf", name="xf")
        for ng in range(2):
            po = psA.tile([128, 512], f32, tag="mm", name="mm")
            for fk in range(16):
                nc.tensor.matmul(po[:], ow[:, fk, :],
                                 ynt[fk // 2][:, (fk % 2) * 1024 + ng * 512:
                                              (fk % 2) * 1024 + (ng + 1) * 512],
                                 start=(fk == 0), stop=(fk == 15))
            t = pool.tile([128, 512], f32, tag="sh_cacc_selu", name="sh_cacc_selu")
            vv = pool.tile([128, 512], f32, tag="svv", name="svv")
            nc.vector.tensor_scalar_min(t[:], po[:], 0.0)
            nc.vector.tensor_tensor(vv[:], po[:], t[:], AL.subtract)
            u = pool.tile([128, 512], f32, tag="sh_cacc_selu", name="sh_cacc_selu")
            nc.scalar.activation(u[:], t[:], AF.Exp)
            nc.vector.scalar_tensor_tensor(
                vv[:], vv[:], SELU_S, P["negsa"][:].broadcast_to([128, 512]),
                AL.mult, AL.add)
            nc.vector.scalar_tensor_tensor(
                nxf[:, ng * 512:(ng + 1) * 512], u[:], SA, vv[:], AL.mult, AL.add)
        new_xf.append(nxf)
    return new_xf


def _build():
    nc = bacc.Bacc("TRN2", target_bir_lowering=False, debug=False)
    dram = {}

    def din(name, shape, dt):
        dram[name] = nc.dram_tensor(name, shape, dt, kind="ExternalInput")

    din("xF", [DM, L], bf16)
    for li in range(2):
        din(f"l{li}_inwT", [DM, DIP], bf16)
        din(f"l{li}_convw", [128, 72], f32)
        din(f"l{li}_convb", [128, 36], f32)
        din(f"l{li}_dtb", [32, 1], f32)
        din(f"l{li}_A", [32, 1], f32)
        din(f"l{li}_Drep", [128, 32], f32)
        din(f"l{li}_outwT", [DI, DM], bf16)
    din("w_head", [DM, 10], bf16)
    din("b_head", [10, 1], f32)
    din("c_id", [128, 128], f32)
    din("c_q2", [128, 64], bf16)
    din("c_pt", [128, 128], bf16)
    din("c_q2blk", [128, 128], bf16)
    din("c_segmask", [32, 1024], f32)
    din("c_negsa", [128, 1], f32)
    out_t = nc.dram_tensor("out", [10, 1], f32, kind="ExternalOutput")

    with tile.TileContext(nc) as tc:
        with (
            tc.tile_pool(name="pool", bufs=2) as pool,
            tc.tile_pool(name="p1", bufs=1) as p1,
            tc.tile_pool(name="xfp", bufs=8) as xfp,
            tc.tile_pool(name="inwp", bufs=8) as inwp,
            tc.tile_pool(name="big8", bufs=8) as big8,
            tc.tile_pool(name="cpool", bufs=1) as cpool,
            tc.tile_pool(name="psA", bufs=2, space="PSUM") as psA,
            tc.tile_pool(name="psB", bufs=1, space="PSUM") as psB,
            tc.tile_pool(name="psC", bufs=2, space="PSUM") as psC,
            tc.tile_pool(name="dramp", bufs=1, space="DRAM") as dramp,
        ):
            id_f32 = cpool.tile([128, 128], f32, tag="id_f32", name="id_f32")
            nc.sync.dma_start(id_f32[:], dram["c_id"].ap())
            id_bf = cpool.tile([128, 128], bf16, tag="id_bf", name="id_bf")
            nc.vector.tensor_copy(id_bf[:], id_f32[:])
            q2 = cpool.tile([128, 64], bf16, tag="q2", name="q2")
            nc.sync.dma_start(q2[:], dram["c_q2"].ap())
            pt_bf = cpool.tile([128, 128], bf16, tag="pt_bf", name="pt_bf")
            nc.sync.dma_start(pt_bf[:], dram["c_pt"].ap())
            q2blk = cpool.tile([128, 128], bf16, tag="q2blk", name="q2blk")
            nc.sync.dma_start(q2blk[:], dram["c_q2blk"].ap())
            segmask = cpool.tile([32, 1024], f32, tag="segmask", name="segmask")
            nc.sync.dma_start(segmask[:], dram["c_segmask"].ap())
            negsa = cpool.tile([128, 1], f32, tag="negsa", name="negsa")
            nc.sync.dma_start(negsa[:], dram["c_negsa"].ap())
            epsb = cpool.tile([128, 1], f32, tag="epsb", name="epsb")
            nc.vector.memset(epsb[:], 1e-5)

            P = dict(pool=pool, p1=p1, xfp=xfp, inwp=inwp, big8=big8,
                     psA=psA, psB=psB, psC=psC, dramp=dramp,
                     id_f32=id_f32, id_bf=id_bf, q2blk=q2blk, epsb=epsb,
                     q2=q2, pt_bf=pt_bf, segmask=segmask, negsa=negsa)

            xf = [xfp.tile([128, 1024], bf16, tag="xf", name="xf") for _ in range(8)]
            for dk in range(8):
                nc.sync.dma_start(xf[dk][:],
                                  dram["xF"].ap()[dk * 128:(dk + 1) * 128, :])

            h1 = _emit_layer(nc, P, dram, 0, xf)
            h2 = _emit_layer(nc, P, dram, 1, h1)

            pooledf = cpool.tile([128, 8], f32, tag="pooledf", name="pooledf")
            pooled = cpool.tile([128, 8], bf16, tag="pooled", name="pooled")
            for dk in range(8):
                nc.vector.tensor_reduce(pooledf[:, dk:dk + 1], h2[dk][:],
                                        AX.X, AL.add)
            nc.vector.tensor_copy(pooled[:], pooledf[:])
            wh = cpool.tile([128, 8, 10], bf16, tag="wh", name="wh")
            nc.sync.dma_start(wh[:], dram["w_head"].ap()
                              .rearrange("(k p) o -> p k o", p=128))
            bh = cpool.tile([10, 1], f32, tag="bh", name="bh")
            nc.sync.dma_start(bh[:], dram["b_head"].ap())
            ph = psC.tile([128, 128], f32, tag="trd", name="trd")
            for dk in range(8):
                nc.tensor.matmul(ph[:10, 0:1], wh[:, dk, :], pooled[:, dk:dk + 1],
                                 start=(dk == 0), stop=(dk == 7))
            osb = cpool.tile([10, 1], f32, tag="osb", name="osb")
            nc.vector.tensor_tensor(osb[:], ph[:10, 0:1], bh[:], AL.add)
            nc.sync.dma_start(out_t.ap(), osb[:])

    nc.compile()
    return nc


def _prep_maps(inputs):
    npbf = mybir.dt.np(bf16)
    x = np.asarray(inputs["x"], np.float32)
    shared = {}
    for li, pre in enumerate(("m1", "m2")):
        inw = np.asarray(inputs[f"{pre}_in_w"], np.float32)
        shared[f"l{li}_inwT"] = np.ascontiguousarray(inw.T).astype(npbf)
        cw = np.asarray(inputs[f"{pre}_conv_w"], np.float32)
        shared[f"l{li}_convw"] = np.ascontiguousarray(
            cw.reshape(18, 128, 4).transpose(1, 0, 2).reshape(128, 72))
        cbt = np.asarray(inputs[f"{pre}_conv_b"], np.float32).reshape(18, 128).T
        shared[f"l{li}_convb"] = np.ascontiguousarray(
            np.concatenate([cbt, -cbt], 1))
        shared[f"l{li}_dtb"] = np.asarray(
            inputs[f"{pre}_dt_bias"], np.float32).reshape(32, 1)
        shared[f"l{li}_A"] = (-np.exp(np.asarray(
            inputs[f"{pre}_A_log"], np.float32))).reshape(32, 1)
        shared[f"l{li}_Drep"] = np.ascontiguousarray(np.broadcast_to(
            np.asarray(inputs[f"{pre}_D"], np.float32)[None, :], (128, 32)))
        ow = np.asarray(inputs[f"{pre}_out_w"], np.float32)
        nw = np.asarray(inputs[f"{pre}_norm_w"], np.float32)
        shared[f"l{li}_outwT"] = np.ascontiguousarray(
            (ow * nw[None, :]).T).astype(npbf)
    wcat = np.concatenate([np.asarray(inputs["w_emo"], np.float32),
                           np.asarray(inputs["w_sen"], np.float32)], 0) / float(L)
    shared["w_head"] = np.ascontiguousarray(wcat.T).astype(npbf)
    shared["b_head"] = np.concatenate(
        [np.asarray(inputs["b_emo"], np.float32),
         np.asarray(inputs["b_sen"], np.float32)]).reshape(10, 1)
    shared["c_id"] = np.eye(128, dtype=np.float32)
    q = (np.arange(64)[:, None] <= np.arange(64)[None, :]).astype(np.float32)
    shared["c_q2"] = np.ascontiguousarray(np.concatenate([q, q], 0)).astype(npbf)
    pt = (np.arange(64)[:, None] > np.arange(64)[None, :]).astype(np.float32)
    z64 = np.zeros((64, 64), np.float32)
    shared["c_pt"] = np.ascontiguousarray(
        np.block([[pt, z64], [z64, pt]])).astype(npbf)
    shared["c_q2blk"] = np.ascontiguousarray(
        np.block([[q, z64], [z64, q]])).astype(npbf)
    sm = np.ones((32, 1024), np.float32)
    sm[:, ::64] = 0.0
    shared["c_segmask"] = sm
    shared["c_negsa"] = np.full((128, 1), -SA, np.float32)

    maps = []
    for b in range(8):
        m = dict(shared)
        m["xF"] = np.ascontiguousarray(x[b].T).astype(npbf)
        maps.append(m)
    return maps


def _get_nc():
    if "nc" not in _CACHE:
        _CACHE["nc"] = _build()
    return _CACHE["nc"]


def kernel(**inputs):
    nc = _get_nc()
    maps = _prep_maps(inputs)
    res = bass_utils.run_bass_kernel_spmd(nc, maps, core_ids=list(range(8)))
    return np.stack([r["out"][:, 0] for r in res.results], 0).astype(np.float32)


# revision 14
# speedup vs baseline: 535.9393x; 535.9393x over previous
"""Trainium2 Bass kernel for nn_AudioModelM3 (2-layer Mamba2 + heads).

Sharding: batch 8 -> 8 NeuronCores (data parallel, replicated weights).
Per core: one [1024(seq), 1024(d_model)] sequence through two Mamba2 blocks,
selu, mean-pool, and the two linear heads -> [10] logits.

Layouts: projections + depthwise conv run feature-major [feature, time];
the SSD chunk math runs time-major per (head, chunk) via one PE transpose of
xs; per-head scalar vectors (dA, decay, exp(Acs), dt) are packed into a
[128, 1024] tile and transposed once. The inter-chunk state scan uses the
DVE tensor_tensor_scan with zeroed decay at segment starts. z and y round-
trip through DRAM to stay inside SBUF.
"""
import numpy as np

import concourse.bacc as bacc
import concourse.bass as bass
import concourse.mybir as mybir
import concourse.tile as tile
from concourse import bass_utils

f32 = mybir.dt.float32
bf16 = mybir.dt.bfloat16
AL = mybir.AluOpType
AF = mybir.ActivationFunctionType
AX = mybir.AxisListType

L, DM, DI, DS, NH, HD = 1024, 1024, 2048, 128, 32, 64
NCH, CH = 16, 64
DIP = 2 * DI + 2 * DS + NH          # 4384
SELU_S = 1.0507009873554805
SELU_A = 1.6732632423543772
SA = SELU_S * SELU_A
HG = 2                               # heads per scan group
NHG = NH // HG

_CACHE = {}


def _emit_layer(nc, P, dram, li, xf_tiles):
    pool, p1, psA, psB, psC = P["pool"], P["p1"], P["psA"], P["psB"], P["psC"]
    big8, xfp = P["big8"], P["xfp"]
    id_bf, id_f32, q2, pt_bf, segmask = (P["id_bf"], P["id_f32"], P["q2"],
                                         P["pt_bf"], P["segmask"])
    z_dram = P["dramp"].tile([L, DI], bf16, tag=f"zdram{li}", name=f"zdram{li}")[:]
    y_dram = P["dramp"].tile([L, DI], bf16, tag=f"ydram{li}", name=f"ydram{li}")[:]

    convw = p1.tile([128, 72], f32, tag="convw", name="convw")
    convb = p1.tile([128, 36], f32, tag="convb", name="convb")
    dtb = p1.tile([32, 1], f32, tag="dtb", name="dtb")
    Acoef = p1.tile([32, 1], f32, tag="Acoef", name="Acoef")
    drep = p1.tile([128, 32], f32, tag="drep", name="drep")
    nc.sync.dma_start(convw[:], dram[f"l{li}_convw"].ap())
    nc.sync.dma_start(convb[:], dram[f"l{li}_convb"].ap())
    nc.sync.dma_start(dtb[:], dram[f"l{li}_dtb"].ap())
    nc.sync.dma_start(Acoef[:], dram[f"l{li}_A"].ap())
    nc.sync.dma_start(drep[:], dram[f"l{li}_Drep"].ap())
    inw_dram = dram[f"l{li}_inwT"].ap()
    outw_dram = dram[f"l{li}_outwT"].ap()

    xtm = [big8.tile([128, 2048], bf16, tag="big8", name="big8") for _ in range(8)]
    bt = p1.tile([128, 1024], bf16, tag="bt", name="bt")
    ct = p1.tile([128, 1024], bf16, tag="ct", name="ct")
    btm2 = p1.tile([128, 16, 128], bf16, tag="btm2", name="btm2")
    nc.vector.memset(btm2[:], 0.0)
    dtraw = pool.tile([32, 1024], f32, tag="sh_dtraw_zt", name="sh_dtraw_zt")

    # ============ Phase A: in_proj (+ conv + xs/B transposes) ============
    for f0, fw in [(k * 512, 512) for k in range(8)] + [(4096, 288)]:
        inw = [P["inwp"].tile([128, 512], bf16, tag="inw", name="inw") for _ in range(8)]
        for dk in range(8):
            nc.sync.dma_start(inw[dk][:, :fw],
                              inw_dram[dk * 128:(dk + 1) * 128, f0:f0 + fw])
        if f0 < 2048:                      # --- z part (time-major) ---
            for lt in range(8):
                pz = psA.tile([128, 512], f32, tag="mm", name="mm")
                for dk in range(8):
                    nc.tensor.matmul(pz[:], xf_tiles[dk][:, lt * 128:(lt + 1) * 128],
                                     inw[dk][:, :512], start=(dk == 0), stop=(dk == 7))
                zsb = pool.tile([128, 512], bf16, tag="zsb", name="zsb")
                nc.any.tensor_copy(zsb[:], pz[:])
                nc.sync.dma_start(z_dram[lt * 128:(lt + 1) * 128, f0:f0 + 512], zsb[:])
        else:                              # --- conv-channel / dt parts ---
            if f0 < 4096:
                fts = [(f0 - 2048) // 128 + j for j in range(4)]
                locs = [j * 128 for j in range(4)]
            else:
                fts, locs = [16, 17], [0, 128]
            for ft, loc in zip(fts, locs):
                pad = pool.tile([128, 1027], bf16, tag="pad", name="pad")
                nc.vector.memset(pad[:, 0:3], 0.0)
                for ng in range(2):
                    px = psA.tile([128, 512], f32, tag="mm", name="mm")
                    for dk in range(8):
                        nc.tensor.matmul(px[:], inw[dk][:, loc:loc + 128],
                                         xf_tiles[dk][:, ng * 512:(ng + 1) * 512],
                                         start=(dk == 0), stop=(dk == 7))
                    nc.any.tensor_copy(pad[:, 3 + ng * 512:3 + (ng + 1) * 512], px[:])
                # causal depthwise conv + bias + silu
                acc = pool.tile([128, 1024], f32, tag="sh_cacc_selu", name="sh_cacc_selu")
                nc.vector.tensor_scalar_mul(acc[:], pad[:, 0:1024],
                                            convw[:, ft * 4:ft * 4 + 1])
                for k in (1, 2, 3):
                    nc.vector.scalar_tensor_tensor(
                        acc[:], pad[:, k:k + 1024],
                        convw[:, ft * 4 + k:ft * 4 + k + 1], acc[:], AL.mult, AL.add)
                if ft < 16:
                    dest = pool.tile([128, 1024], bf16, tag="xsf", name="xsf")
                elif ft == 16:
                    dest = bt
                else:
                    dest = ct
                for q in range(2):
                    sl = slice(q * 512, (q + 1) * 512)
                    u = pool.tile([128, 512], bf16, tag="cu", name="cu")
                    e = pool.tile([128, 512], f32, tag="ce", name="ce")
                    nc.scalar.activation(u[:], acc[:, sl], AF.Identity,
                                         bias=convb[:, ft:ft + 1], scale=1.0)
                    nc.scalar.activation(e[:], acc[:, sl], AF.Exp,
                                         bias=convb[:, 18 + ft:19 + ft], scale=-1.0)
                    nc.vector.tensor_scalar_add(e[:], e[:], 1.0)
                    r = pool.tile([128, 512], f32, tag="ce", name="ce")
                    nc.vector.reciprocal(r[:], e[:])
                    nc.vector.tensor_tensor(dest[:, sl], u[:], r[:], AL.mult)
                if ft < 16:
                    for p in range(8):
                        ptr = psC.tile([128, 128], bf16, tag="trb", name="trb")
                        nc.tensor.transpose(ptr[:], dest[:, p * 128:(p + 1) * 128],
                                            id_bf[:])
                        nc.any.tensor_copy(xtm[p][:, ft * 128:(ft + 1) * 128], ptr[:])
                elif ft == 16:
                    for c in range(NCH):
                        ptr = psC.tile([128, 128], bf16, tag="trb", name="trb")
                        half = (c % 2) * 64
                        nc.tensor.transpose(ptr[half:half + 64, :],
                                            bt[:, c * 64:(c + 1) * 64], id_bf[:])
                        nc.any.tensor_copy(btm2[half:half + 64, c, :],
                                       ptr[half:half + 64, :])
            if f0 >= 4096:                 # --- dt part ---
                for ng in range(2):
                    pd = psA.tile([128, 512], f32, tag="mm", name="mm")
                    for dk in range(8):
                        nc.tensor.matmul(pd[:32, :], inw[dk][:, 256:288],
                                         xf_tiles[dk][:, ng * 512:(ng + 1) * 512],
                                         start=(dk == 0), stop=(dk == 7))
                    nc.any.tensor_copy(dtraw[:, ng * 512:(ng + 1) * 512], pd[:32, :])

    # ============ Phase C: dt pipeline -> packed time-major scalars ======
    tmp1 = p1.tile([32, 1024], f32, tag="dttmp1", name="dttmp1")
    dt_t = p1.tile([32, 1024], f32, tag="dt_t", name="dt_t")
    dA_t = p1.tile([32, 1024], f32, tag="dA_t", name="dA_t")
    dfac_t = p1.tile([32, 1024], f32, tag="dfac_t", name="dfac_t")
    eacs_t = p1.tile([32, 1024], f32, tag="eacs_t", name="eacs_t")
    nc.scalar.activation(tmp1[:], dtraw[:], AF.Exp, bias=dtb[:], scale=1.0)
    nc.scalar.activation(dt_t[:], tmp1[:], AF.Ln, bias=1.0, scale=1.0)
    nc.vector.tensor_scalar_mul(dA_t[:], dt_t[:], Acoef[:])
    acs = pool.tile([32, 1024], f32, tag="sh_acs_outw", name="sh_acs_outw")
    nc.vector.tensor_tensor_scan(acs[:], segmask[:], dA_t[:], 0.0,
                                 AL.mult, AL.add)
    lastb = acs[:, 63::64].unsqueeze(2).broadcast_to([32, 16, 64])
    d0 = p1.tile([32, 1024], f32, tag="dttmp2", name="dttmp2")
    nc.vector.tensor_tensor(d0[:].rearrange("p (c q) -> p c q", c=16), lastb,
                            acs[:].rearrange("p (c q) -> p c q", c=16), AL.subtract)
    nc.scalar.activation(tmp1[:], d0[:], AF.Exp)
    nc.vector.tensor_tensor(dfac_t[:], tmp1[:], dt_t[:], AL.mult)
    nc.scalar.activation(eacs_t[:], acs[:], AF.Exp)
    # transpose the four [32,1024] quantity maps to time-major [128, 8, 32]
    tmq = []
    for qi, qt in enumerate((dA_t, dfac_t, eacs_t, dt_t)):
        tq = p1.tile([128, 8, 32], f32, tag=f"tmq{qi}", name=f"tmq{qi}")
        for i in range(8):
            ptr = psC.tile([128, 128], f32, tag="trd", name="trd")
            nc.tensor.transpose(ptr[:, 0:32], qt[:, i * 128:(i + 1) * 128],
                                id_f32[:32, :32])
            nc.any.tensor_copy(tq[:, i, :], ptr[:, 0:32])
        tmq.append(tq)
    tm_dA, tm_dfac, tm_eacs, tm_dt = tmq
    decrow = p1.tile([1, 512], f32, tag="decrow", name="decrow")
    nc.sync.dma_start(decrow[:].rearrange("o (h c) -> o h c", h=32),
                      eacs_t[:, 63::64])
    dec_rep = p1.tile([128, 512], f32, tag="dec_rep", name="dec_rep")
    nc.gpsimd.partition_broadcast(dec_rep[:], decrow[:])
    nc.vector.memset(dec_rep[:, 0:512:16], 0.0)

    # ====== GT: masked chunk grams, block-diag pair layout [128,(p,128)] ==
    gtmblk = p1.tile([128, 1024], f32, tag="gtmblk", name="gtmblk")
    pg = psB.tile([128, 1024], f32, tag="wide", name="wide")
    nc.vector.memset(pg[:], 0.0)
    for c in range(NCH):
        p, half = c // 2, (c % 2) * 64
        nc.tensor.matmul(pg[half:half + 64, p * 128 + half:p * 128 + half + 64],
                         bt[:, c * 64:(c + 1) * 64], ct[:, c * 64:(c + 1) * 64],
                         start=True, stop=True)
    ub = P["q2blk"][:].unsqueeze(1).broadcast_to([128, 8, 128])
    nc.vector.tensor_tensor(gtmblk[:].rearrange("p (c q) -> p c q", c=8),
                            pg[:].rearrange("p (c q) -> p c q", c=8), ub, AL.mult)

    # ============ Phase D: SSD per head-group ============================
    for hg in range(NHG):
        scan_in = p1.tile([128, HG * 1024], f32, tag="scan_in", name="scan_in")
        scan_out = p1.tile([128, HG * 1024], bf16, tag="scan_out", name="scan_out")
        mt_all = p1.tile([128, HG * 1024], bf16, tag="mt_all", name="mt_all")
        xcdt_all = p1.tile([128, HG * 512], bf16, tag="xcdt", name="xcdt")
        for hl in range(HG):
            h = hg * HG + hl
            daq = pool.tile([128, 512], bf16, tag="daq", name="daq")
            xcdec = pool.tile([128, 512], bf16, tag="xcdec", name="xcdec")
            for p in range(8):
                nc.vector.tensor_scalar_mul(daq[:, p * 64:(p + 1) * 64], q2[:],
                                            tm_dA[:, p, h:h + 1])
                nc.vector.tensor_scalar_mul(
                    xcdt_all[:, hl * 512 + p * 64:hl * 512 + (p + 1) * 64],
                    xtm[p][:, h * 64:(h + 1) * 64], tm_dt[:, p, h:h + 1])
                nc.vector.tensor_scalar_mul(xcdec[:, p * 64:(p + 1) * 64],
                                            xtm[p][:, h * 64:(h + 1) * 64],
                                            tm_dfac[:, p, h:h + 1])
            pdif = psB.tile([128, 1024], f32, tag="wide", name="wide")
            nc.vector.memset(pdif[:], 0.0)
            for half in (0, 64):
                oap = pdif[half:half + 64, :].rearrange(
                    "s (p l) -> s p l", p=8)[:, :, half:half + 64]
                nc.tensor.matmul(oap, pt_bf[:, half:half + 64], daq[:],
                                 start=True, stop=True)
            ll = pool.tile([128, 1024], bf16, tag="ll", name="ll")
            nc.scalar.activation(ll[:], pdif[:], AF.Exp)
            nc.vector.tensor_tensor(mt_all[:, hl * 1024:(hl + 1) * 1024],
                                    ll[:], gtmblk[:], AL.mult)
            for grp in range(2):
                pst = psA.tile([128, 512], f32, tag="mm", name="mm")
                for j in range(8):
                    c = grp * 8 + j
                    nc.tensor.matmul(
                        pst[:, j * 64:(j + 1) * 64], btm2[:, c, :],
                        xcdec[:, (c // 2) * 64:(c // 2) * 64 + 64],
                        start=True, stop=True)
                out_ap = scan_in[:].rearrange("n (h d c) -> n h d c", h=HG, d=HD)[
                    :, hl, :, grp * 8:(grp + 1) * 8].transpose([0, 2, 1])
                nc.vector.tensor_copy(out_ap,
                                      pst[:].rearrange("n (c d) -> n c d", c=8))
            decb = p1.tile([128, 1024], f32, tag="decb", name="decb")
            nc.vector.tensor_copy(
                decb[:].rearrange("n (d c) -> n d c", d=HD),
                dec_rep[:, h * 16:(h + 1) * 16].unsqueeze(1)
                .broadcast_to([128, HD, 16]))
            nc.vector.tensor_tensor_scan(
                scan_out[:, hl * 1024:(hl + 1) * 1024], decb[:],
                scan_in[:, hl * 1024:(hl + 1) * 1024], 0.0, AL.mult, AL.add)

        # ---- Yo + Yd + D-skip combine, per chunk-pair ----
        for p in range(8):
            pyo = psA.tile([128, 512], f32, tag="mm", name="mm")
            pyd = psA.tile([128, 512], f32, tag="mm", name="mm")
            W = HG * HD                      # 128 cols
            for half in range(2):
                c = 2 * p + half
                osl = slice(half * 64, half * 64 + 64)
                if c == 0:
                    nc.vector.memset(pyo[osl, 0:W], 0.0)
                else:
                    rhs = scan_out[:].rearrange("n (h d c) -> n h d c",
                                                h=HG, d=HD)[:, :, :, c - 1]
                    nc.tensor.matmul(pyo[osl, 0:W], ct[:, c * 64:(c + 1) * 64],
                                     rhs, start=True, stop=True)
            for hl in range(HG):
                nc.tensor.matmul(
                    pyd[:, hl * 64:(hl + 1) * 64],
                    mt_all[:, hl * 1024 + p * 128:hl * 1024 + (p + 1) * 128],
                    xcdt_all[:, hl * 512 + p * 64:hl * 512 + (p + 1) * 64],
                    start=True, stop=True)
            eb = tm_eacs[:, p, hg * HG:(hg + 1) * HG].unsqueeze(2) \
                .broadcast_to([128, HG, HD])
            t1 = pool.tile([128, 128], f32, tag="comb1", name="comb1")
            nc.vector.tensor_tensor(t1[:].rearrange("n (h d) -> n h d", h=HG),
                                    pyo[:, 0:W].rearrange("n (h d) -> n h d", h=HG),
                                    eb, AL.mult)
            t2 = pool.tile([128, 128], f32, tag="comb2", name="comb2")
            nc.vector.tensor_tensor(t2[:], t1[:], pyd[:, 0:W], AL.add)
            db = drep[:, hg * HG:(hg + 1) * HG].unsqueeze(2) \
                .broadcast_to([128, HG, HD])
            t3 = pool.tile([128, 128], f32, tag="comb3", name="comb3")
            nc.vector.tensor_tensor(
                t3[:].rearrange("n (h d) -> n h d", h=HG),
                xtm[p][:, hg * W:(hg + 1) * W].rearrange("n (h d) -> n h d", h=HG),
                db, AL.mult)
            yout = pool.tile([128, 128], bf16, tag="yout", name="yout")
            nc.vector.tensor_tensor(yout[:], t2[:], t3[:], AL.add)
            nc.sync.dma_start(y_dram[p * 128:(p + 1) * 128, hg * W:(hg + 1) * W],
                              yout[:])

    # ============ Phase E: gating + rmsnorm + y transpose ================
    ynt = [big8.tile([128, 2048], bf16, tag="big8", name="big8") for _ in range(8)]
    for p in range(8):
        zt = pool.tile([128, 2048], bf16, tag="sh_dtraw_zt", name="sh_dtraw_zt")
        nc.sync.dma_start(zt[:], z_dram[p * 128:(p + 1) * 128, :])
        yin = pool.tile([128, 2048], bf16, tag="yin", name="yin")
        nc.sync.dma_start(yin[:], y_dram[p * 128:(p + 1) * 128, :])
        yg = pool.tile([128, 2048], bf16, tag="sh_pk_yg", name="sh_pk_yg")
        ss = pool.tile([128, 4], f32, tag="ss", name="ss")
        for q in range(4):
            sl = slice(q * 512, (q + 1) * 512)
            e = pool.tile([128, 512], bf16, tag="ge", name="ge")
            nc.scalar.activation(e[:], zt[:, sl], AF.Exp, scale=-1.0)
            nc.vector.tensor_scalar_add(e[:], e[:], 1.0)
            r = pool.tile([128, 512], f32, tag="gr", name="gr")
            nc.vector.reciprocal(r[:], e[:])
            sg = pool.tile([128, 512], bf16, tag="ge", name="ge")
            nc.vector.tensor_tensor(sg[:], zt[:, sl], r[:], AL.mult)
            nc.vector.tensor_tensor(yg[:, sl], yin[:, sl], sg[:], AL.mult)
            sqs = pool.tile([128, 512], f32, tag="gr", name="gr")
            nc.scalar.activation(sqs[:], yg[:, sl], AF.Square,
                                 accum_out=ss[:, q:q + 1])
        sstot = pool.tile([128, 2], f32, tag="sstot", name="sstot")
        nc.vector.tensor_reduce(sstot[:, 0:1], ss[:], AX.X, AL.add)
        nc.scalar.activation(sstot[:, 1:2], sstot[:, 0:1], AF.Ln,
                             bias=P["epsb"][:], scale=1.0 / DI)
        r2 = pool.tile([128, 1], f32, tag="r2", name="r2")
        nc.scalar.activation(r2[:], sstot[:, 1:2], AF.Exp, scale=-0.5)
        diag = pool.tile([128, 128], bf16, tag="diag", name="diag")
        nc.vector.tensor_scalar_mul(diag[:], id_bf[:], r2[:])
        for fk in range(16):
            ptr = psC.tile([128, 128], f32, tag="trd", name="trd")
            nc.tensor.matmul(ptr[:], yg[:, fk * 128:(fk + 1) * 128], diag[:],
                             start=True, stop=True)
            nc.any.tensor_copy(ynt[fk // 2][:, (fk % 2) * 1024 + p * 128:
                                        (fk % 2) * 1024 + (p + 1) * 128], ptr[:])

    # ============ Phase G: out_proj + selu -> new feature-major x ========
    new_xf = []
    for dk in range(8):
        ow = pool.tile([128, 16, 128], bf16, tag="sh_acs_outw", name="sh_acs_outw")
        nc.sync.dma_start(ow[:], outw_dram[:, dk * 128:(dk + 1) * 128]
                          .rearrange("(f p) d -> p f d", p=128))
        nxf = xfp.tile([128, 1024], bf16, tag="xf", name="xf")
        for ng in range(2):
            po = psA.tile([128, 512], f32, tag="mm", name="mm")
            for fk in range(16):
                nc.tensor.matmul(po[:], ow[:, fk, :],
                                 ynt[fk // 2][:, (fk % 2) * 1024 + ng * 512:
                                              (fk % 2) * 1024 + (ng + 1) * 512],
                                 start=(fk == 0), stop=(fk == 15))
            t = pool.tile([128, 512], f32, tag="sh_cacc_selu", name="sh_cacc_selu")
            vv = pool.tile([128, 512], f32, tag="svv", name="svv")
            nc.vector.tensor_scalar_min(t[:], po[:], 0.0)
            nc.vector.tensor_tensor(vv[:], po[:], t[:], AL.subtract)
            u = pool.tile([128, 512], f32, tag="sh_cacc_selu", name="sh_cacc_selu")
            nc.scalar.activation(u[:], t[:], AF.Exp)
            nc.vector.scalar_tensor_tensor(
                vv[:], vv[:], SELU_S, P["negsa"][:].broadcast_to([128, 512]),
                AL.mult, AL.add)
            nc.vector.scalar_tensor_tensor(
                nxf[:, ng * 512:(ng + 1) * 512], u[:], SA, vv[:], AL.mult, AL.add)
        new_xf.append(nxf)
    return new_xf


def _build():
    nc = bacc.Bacc("TRN2", target_bir_lowering=False, debug=False)
    dram = {}

    def din(name, shape, dt):
        dram[name] = nc.dram_tensor(name, shape, dt, kind="ExternalInput")

    din("xF", [DM, L], bf16)
    for li in range(2):
        din(f"l{li}_inwT", [DM, DIP], bf16)
        din(f"l{li}_convw", [128, 72], f32)
        din(f"l{li}_convb", [128, 36], f32)
        din(f"l{li}_dtb", [32, 1], f32)
        din(f"l{li}_A", [32, 1], f32)
        din(f"l{li}_Drep", [128, 32], f32)
        din(f"l{li}_outwT", [DI, DM], bf16)
    din("w_head", [DM, 10], bf16)
    din("b_head", [10, 1], f32)
    din("c_id", [128, 128], f32)
    din("c_q2", [128, 64], bf16)
    din("c_pt", [128, 128], bf16)
    din("c_q2blk", [128, 128], bf16)
    din("c_segmask", [32, 1024], f32)
    din("c_negsa", [128, 1], f32)
    out_t = nc.dram_tensor("out", [10, 1], f32, kind="ExternalOutput")

    with tile.TileContext(nc) as tc:
        with (
            tc.tile_pool(name="pool", bufs=2) as pool,
            tc.tile_pool(name="p1", bufs=1) as p1,
            tc.tile_pool(name="xfp", bufs=8) as xfp,
            tc.tile_pool(name="inwp", bufs=8) as inwp,
            tc.tile_pool(name="big8", bufs=8) as big8,
            tc.tile_pool(name="cpool", bufs=1) as cpool,
            tc.tile_pool(name="psA", bufs=2, space="PSUM") as psA,
            tc.tile_pool(name="psB", bufs=1, space="PSUM") as psB,
            tc.tile_pool(name="psC", bufs=2, space="PSUM") as psC,
            tc.tile_pool(name="dramp", bufs=1, space="DRAM") as dramp,
        ):
            id_f32 = cpool.tile([128, 128], f32, tag="id_f32", name="id_f32")
            nc.sync.dma_start(id_f32[:], dram["c_id"].ap())
            id_bf = cpool.tile([128, 128], bf16, tag="id_bf", name="id_bf")
            nc.vector.tensor_copy(id_bf[:], id_f32[:])
            q2 = cpool.tile([128, 64], bf16, tag="q2", name="q2")
            nc.sync.dma_start(q2[:], dram["c_q2"].ap())
            pt_bf = cpool.tile([128, 128], bf16, tag="pt_bf", name="pt_bf")
            nc.sync.dma_start(pt_bf[:], dram["c_pt"].ap())
            q2blk = cpool.tile([128, 128], bf16, tag="q2blk", name="q2blk")
            nc.sync.dma_start(q2blk[:], dram["c_q2blk"].ap())
            segmask = cpool.tile([32, 1024], f32, tag="segmask", name="segmask")
            nc.sync.dma_start(segmask[:], dram["c_segmask"].ap())
            negsa = cpool.tile([128, 1], f32, tag="negsa", name="negsa")
            nc.sync.dma_start(negsa[:], dram["c_negsa"].ap())
            epsb = cpool.tile([128, 1], f32, tag="epsb", name="epsb")
            nc.vector.memset(epsb[:], 1e-5)

            P = dict(pool=pool, p1=p1, xfp=xfp, inwp=inwp, big8=big8,
                     psA=psA, psB=psB, psC=psC, dramp=dramp,
                     id_f32=id_f32, id_bf=id_bf, q2blk=q2blk, epsb=epsb,
                     q2=q2, pt_bf=pt_bf, segmask=segmask, negsa=negsa)

            xf = [xfp.tile([128, 1024], bf16, tag="xf", name="xf") for _ in range(8)]
            for dk in range(8):
                nc.sync.dma_start(xf[dk][:],
                                  dram["xF"].ap()[dk * 128:(dk + 1) * 128, :])

            h1 = _emit_layer(nc, P, dram, 0, xf)
            h2 = _emit_layer(nc, P, dram, 1, h1)

            pooledf = cpool.tile([128, 8], f32, tag="pooledf", name="pooledf")
            pooled = cpool.tile([128, 8], bf16, tag="pooled", name="pooled")
            for dk in range(8):
                nc.vector.tensor_reduce(pooledf[:, dk:dk + 1], h2[dk][:],
                                        AX.X, AL.add)
            nc.vector.tensor_copy(pooled[:], pooledf[:])
            wh = cpool.tile([128, 8, 10], bf16, tag="wh", name="wh")
            nc.sync.dma_start(wh[:], dram["w_head"].ap()
                              .rearrange("(k p) o -> p k o", p=128))
            bh = cpool.tile([10, 1], f32, tag="bh", name="bh")
            nc.sync.dma_start(bh[:], dram["b_head"].ap())
            ph = psC.tile([128, 128], f32, tag="trd", name="trd")
            for dk in range(8):
                nc.tensor.matmul(ph[:10, 0:1], wh[:, dk, :], pooled[:, dk:dk + 1],
                                 start=(dk == 0), stop=(dk == 7))
            osb = cpool.tile([10, 1], f32, tag="osb", name="osb")
            nc.vector.tensor_tensor(osb[:], ph[:10, 0:1], bh[:], AL.add)
            nc.sync.dma_start(out_t.ap(), osb[:])

    nc.compile()
    return nc


def _prep_maps(inputs):
    npbf = mybir.dt.np(bf16)
    x = np.asarray(inputs["x"], np.float32)
    shared = {}
    for li, pre in enumerate(("m1", "m2")):
        inw = np.asarray(inputs[f"{pre}_in_w"], np.float32)
        shared[f"l{li}_inwT"] = np.ascontiguousarray(inw.T).astype(npbf)
        cw = np.asarray(inputs[f"{pre}_conv_w"], np.float32)
        shared[f"l{li}_convw"] = np.ascontiguousarray(
            cw.reshape(18, 128, 4).transpose(1, 0, 2).reshape(128, 72))
        cbt = np.asarray(inputs[f"{pre}_conv_b"], np.float32).reshape(18, 128).T
        shared[f"l{li}_convb"] = np.ascontiguousarray(
            np.concatenate([cbt, -cbt], 1))
        shared[f"l{li}_dtb"] = np.asarray(
            inputs[f"{pre}_dt_bias"], np.float32).reshape(32, 1)
        shared[f"l{li}_A"] = (-np.exp(np.asarray(
            inputs[f"{pre}_A_log"], np.float32))).reshape(32, 1)
        shared[f"l{li}_Drep"] = np.ascontiguousarray(np.broadcast_to(
            np.asarray(inputs[f"{pre}_D"], np.float32)[None, :], (128, 32)))
        ow = np.asarray(inputs[f"{pre}_out_w"], np.float32)
        nw = np.asarray(inputs[f"{pre}_norm_w"], np.float32)
        shared[f"l{li}_outwT"] = np.ascontiguousarray(
            (ow * nw[None, :]).T).astype(npbf)
    wcat = np.concatenate([np.asarray(inputs["w_emo"], np.float32),
                           np.asarray(inputs["w_sen"], np.float32)], 0) / float(L)
    shared["w_head"] = np.ascontiguousarray(wcat.T).astype(npbf)
    shared["b_head"] = np.concatenate(
        [np.asarray(inputs["b_emo"], np.float32),
         np.asarray(inputs["b_sen"], np.float32)]).reshape(10, 1)
    shared["c_id"] = np.eye(128, dtype=np.float32)
    q = (np.arange(64)[:, None] <= np.arange(64)[None, :]).astype(np.float32)
    shared["c_q2"] = np.ascontiguousarray(np.concatenate([q, q], 0)).astype(npbf)
    pt = (np.arange(64)[:, None] > np.arange(64)[None, :]).astype(np.float32)
    z64 = np.zeros((64, 64), np.float32)
    shared["c_pt"] = np.ascontiguousarray(
        np.block([[pt, z64], [z64, pt]])).astype(npbf)
    shared["c_q2blk"] = np.ascontiguousarray(
        np.block([[q, z64], [z64, q]])).astype(npbf)
    sm = np.ones((32, 1024), np.float32)
    sm[:, ::64] = 0.0
    shared["c_segmask"] = sm
    shared["c_negsa"] = np.full((128, 1), -SA, np.float32)

    maps = []
    for b in range(8):
        m = dict(shared)
        m["xF"] = np.ascontiguousarray(x[b].T).astype(npbf)
        maps.append(m)
    return maps


def _get_nc():
    if "nc" not in _CACHE:
        _CACHE["nc"] = _build()
    return _CACHE["nc"]


def _make_runner(nc):
    """Build a persistent jitted 8-core executor (compiles once)."""
    import jax
    import concourse.mybir as _mybir
    from concourse import bass2jax
    from jax.experimental.shard_map import shard_map
    from jax.sharding import Mesh, PartitionSpec

    bass2jax.install_neuronx_cc_hook()
    in_names, out_names, out_avals, zero_outs = [], [], [], []
    for alloc in nc.m.functions[0].allocations:
        if not isinstance(alloc, _mybir.MemoryLocationSet):
            continue
        name = alloc.memorylocations[0].name
        if alloc.kind == "ExternalInput":
            if nc.partition_id_tensor is not None and \
                    name == nc.partition_id_tensor.name:
                continue
            in_names.append(name)
        elif alloc.kind == "ExternalOutput":
            out_names.append(name)
            shape = tuple(alloc.tensor_shape)
            dtype = _mybir.dt.np(alloc.dtype)
            out_avals.append(jax.core.ShapedArray(shape, dtype))
            zero_outs.append(np.zeros(shape, dtype))
    n_params, n_outs = len(in_names), len(out_avals)
    all_names = in_names + out_names
    pid_name = nc.partition_id_tensor.name if nc.partition_id_tensor else None
    if pid_name is not None:
        all_names = all_names + [pid_name]

    def _body(*args):
        operands = list(args)
        if pid_name is not None:
            operands.append(bass2jax.partition_id_tensor())
        outs = bass2jax._bass_exec_p.bind(
            *operands, out_avals=tuple(out_avals), in_names=tuple(all_names),
            out_names=tuple(out_names), lowering_input_output_aliases=(),
            sim_require_finite=True, sim_require_nnan=True, nc=nc)
        return tuple(outs)

    devices = jax.devices()[:8]
    mesh = Mesh(np.asarray(devices), ("core",))
    sharded = jax.jit(
        shard_map(_body, mesh=mesh,
                  in_specs=(PartitionSpec("core"),) * (n_params + n_outs),
                  out_specs=(PartitionSpec("core"),) * n_outs,
                  check_rep=False),
        donate_argnums=tuple(range(n_params, n_params + n_outs)),
        keep_unused=True)

    from jax.sharding import NamedSharding
    sh = NamedSharding(mesh, PartitionSpec("core"))

    def run(maps, iters=1, time_it=False):
        import time as _time
        concat_in = [
            jax.device_put(
                np.concatenate([np.asarray(maps[c][nm]) for c in range(8)], 0), sh)
            for nm in in_names]
        jax.block_until_ready(concat_in)

        def once():
            cz = [np.zeros((8 * z.shape[0], *z.shape[1:]), z.dtype)
                  for z in zero_outs]
            return sharded(*concat_in, *cz)
        outs = jax.block_until_ready(once())
        t = None
        if iters > 1:
            t0 = _time.time()
            for _ in range(iters - 1):
                outs = once()
            jax.block_until_ready(outs)
            t = (_time.time() - t0) / (iters - 1)
        outs = [np.asarray(o) for o in outs]
        res = [{nm: outs[i].reshape(8, *out_avals[i].shape)[c]
                for i, nm in enumerate(out_names)} for c in range(8)]
        return (res, t) if time_it else res

    return run


def _get_runner():
    if "runner" not in _CACHE:
        _CACHE["runner"] = _make_runner(_get_nc())
    return _CACHE["runner"]


def kernel(**inputs):
    maps = _prep_maps(inputs)
    res = _get_runner()(maps)
    return np.stack([r["out"][:, 0] for r in res], 0).astype(np.float32)


# revision 19
# speedup vs baseline: 547.8837x; 1.0223x over previous
"""Trainium2 Bass kernel for nn_AudioModelM3 (2-layer Mamba2 + heads).

Sharding: batch 8 -> 8 NeuronCores (data parallel, replicated weights).
Per core: one [1024(seq), 1024(d_model)] sequence through two Mamba2 blocks,
selu, mean-pool, and the two linear heads -> [10] logits.

Layouts: projections + depthwise conv run feature-major [feature, time];
the SSD chunk math runs time-major per (head, chunk) via one PE transpose of
xs; per-head scalar vectors (dA, decay, exp(Acs), dt) are packed into a
[128, 1024] tile and transposed once. The inter-chunk state scan uses the
DVE tensor_tensor_scan with zeroed decay at segment starts. z and y round-
trip through DRAM to stay inside SBUF.
"""
import numpy as np

import concourse.bacc as bacc
import concourse.bass as bass
import concourse.mybir as mybir
import concourse.tile as tile
from concourse import bass_utils

f32 = mybir.dt.float32
bf16 = mybir.dt.bfloat16
AL = mybir.AluOpType
AF = mybir.ActivationFunctionType
AX = mybir.AxisListType

L, DM, DI, DS, NH, HD = 1024, 1024, 2048, 128, 32, 64
NCH, CH = 16, 64
DIP = 2 * DI + 2 * DS + NH          # 4384
SELU_S = 1.0507009873554805
SELU_A = 1.6732632423543772
SA = SELU_S * SELU_A
HG = 2                               # heads per scan group
NHG = NH // HG

_CACHE = {}


def _emit_layer(nc, P, dram, li, xf_tiles):
    pool, p1, psA, psB, psC = P["pool"], P["p1"], P["psA"], P["psB"], P["psC"]
    xfp = P["xfp"]
    id_bf, id_f32, q2, pt_bf, segmask = (P["id_bf"], P["id_f32"], P["q2"],
                                         P["pt_bf"], P["segmask"])
    z_dram = P["dramp"].tile([L, DI], bf16, tag=f"zdram{li}", name=f"zdram{li}")[:]
    y_dram = P["dramp"].tile([L, DI], bf16, tag=f"ydram{li}", name=f"ydram{li}")[:]

    convw = p1.tile([128, 72], f32, tag="convw", name="convw")
    convb = p1.tile([128, 36], f32, tag="convb", name="convb")
    dtb = p1.tile([32, 1], f32, tag="dtb", name="dtb")
    Acoef = p1.tile([32, 1], f32, tag="Acoef", name="Acoef")
    drep = p1.tile([128, 32], f32, tag="drep", name="drep")
    nc.sync.dma_start(convw[:], dram[f"l{li}_convw"].ap())
    nc.sync.dma_start(convb[:], dram[f"l{li}_convb"].ap())
    nc.sync.dma_start(dtb[:], dram[f"l{li}_dtb"].ap())
    nc.sync.dma_start(Acoef[:], dram[f"l{li}_A"].ap())
    nc.sync.dma_start(drep[:], dram[f"l{li}_Drep"].ap())
    inw_dram = dram[f"l{li}_inwT"].ap()
    outw_dram = dram[f"l{li}_outwT"].ap()

    xtm_all = p1.tile([128, 8, 2048], bf16, tag="bigT", name="bigT")
    bt = p1.tile([128, 1024], bf16, tag="bt", name="bt")
    ct = p1.tile([128, 1024], bf16, tag="ct", name="ct")
    btm2 = p1.tile([128, 16, 128], bf16, tag="btm2", name="btm2")
    nc.vector.memset(btm2[:], 0.0)
    dtraw = pool.tile([32, 1024], f32, tag="sh_dtraw_zt", name="sh_dtraw_zt")

    # ============ Phase A: in_proj (+ conv + xs/B transposes) ============
    for f0, fw in [(k * 512, 512) for k in range(8)] + [(4096, 288)]:
        inw = [P["inwp"].tile([128, 512], bf16, tag="inw", name="inw") for _ in range(8)]
        for dk in range(8):
            nc.sync.dma_start(inw[dk][:, :fw],
                              inw_dram[dk * 128:(dk + 1) * 128, f0:f0 + fw])
        if f0 < 2048:                      # --- z part (time-major) ---
            for lt in range(8):
                pz = psA.tile([128, 512], f32, tag="mm", name="mm")
                for dk in range(8):
                    nc.tensor.matmul(pz[:], xf_tiles[dk][:, lt * 128:(lt + 1) * 128],
                                     inw[dk][:, :512], start=(dk == 0), stop=(dk == 7))
                zsb = pool.tile([128, 512], bf16, tag="zsb", name="zsb")
                nc.any.tensor_copy(zsb[:], pz[:])
                nc.sync.dma_start(z_dram[lt * 128:(lt + 1) * 128, f0:f0 + 512], zsb[:])
        else:                              # --- conv-channel / dt parts ---
            if f0 < 4096:
                fts = [(f0 - 2048) // 128 + j for j in range(4)]
                locs = [j * 128 for j in range(4)]
            else:
                fts, locs = [16, 17], [0, 128]
            for ft, loc in zip(fts, locs):
                pad = pool.tile([128, 1027], bf16, tag="pad", name="pad")
                nc.vector.memset(pad[:, 0:3], 0.0)
                for ng in range(2):
                    px = psA.tile([128, 512], f32, tag="mm", name="mm")
                    for dk in range(8):
                        nc.tensor.matmul(px[:], inw[dk][:, loc:loc + 128],
                                         xf_tiles[dk][:, ng * 512:(ng + 1) * 512],
                                         start=(dk == 0), stop=(dk == 7))
                    nc.any.tensor_copy(pad[:, 3 + ng * 512:3 + (ng + 1) * 512], px[:])
                # causal depthwise conv + bias + silu
                acc = pool.tile([128, 1024], f32, tag="sh_cacc_selu", name="sh_cacc_selu")
                nc.vector.tensor_scalar_mul(acc[:], pad[:, 0:1024],
                                            convw[:, ft * 4:ft * 4 + 1])
                for k in (1, 2, 3):
                    nc.vector.scalar_tensor_tensor(
                        acc[:], pad[:, k:k + 1024],
                        convw[:, ft * 4 + k:ft * 4 + k + 1], acc[:], AL.mult, AL.add)
                if ft < 16:
                    dest = pool.tile([128, 1024], bf16, tag="xsf", name="xsf")
                elif ft == 16:
                    dest = bt
                else:
                    dest = ct
                for q in range(2):
                    sl = slice(q * 512, (q + 1) * 512)
                    u = pool.tile([128, 512], bf16, tag="cu", name="cu")
                    e = pool.tile([128, 512], f32, tag="ce", name="ce")
                    nc.scalar.activation(u[:], acc[:, sl], AF.Identity,
                                         bias=convb[:, ft:ft + 1], scale=1.0)
                    nc.scalar.activation(e[:], acc[:, sl], AF.Exp,
                                         bias=convb[:, 18 + ft:19 + ft], scale=-1.0)
                    nc.vector.tensor_scalar_add(e[:], e[:], 1.0)
                    r = pool.tile([128, 512], f32, tag="ce", name="ce")
                    nc.vector.reciprocal(r[:], e[:])
                    nc.vector.tensor_tensor(dest[:, sl], u[:], r[:], AL.mult)
                if ft < 16:
                    for p in range(8):
                        ptr = psC.tile([128, 128], bf16, tag="trb", name="trb")
                        nc.tensor.transpose(ptr[:], dest[:, p * 128:(p + 1) * 128],
                                            id_bf[:])
                        nc.any.tensor_copy(xtm_all[:, p, ft * 128:(ft + 1) * 128], ptr[:])
                elif ft == 16:
                    for c in range(NCH):
                        ptr = psC.tile([128, 128], bf16, tag="trb", name="trb")
                        half = (c % 2) * 64
                        nc.tensor.transpose(ptr[half:half + 64, :],
                                            bt[:, c * 64:(c + 1) * 64], id_bf[:])
                        nc.any.tensor_copy(btm2[half:half + 64, c, :],
                                       ptr[half:half + 64, :])
            if f0 >= 4096:                 # --- dt part ---
                for ng in range(2):
                    pd = psA.tile([128, 512], f32, tag="mm", name="mm")
                    for dk in range(8):
                        nc.tensor.matmul(pd[:32, :], inw[dk][:, 256:288],
                                         xf_tiles[dk][:, ng * 512:(ng + 1) * 512],
                                         start=(dk == 0), stop=(dk == 7))
                    nc.any.tensor_copy(dtraw[:, ng * 512:(ng + 1) * 512], pd[:32, :])

    # ============ Phase C: dt pipeline -> packed time-major scalars ======
    tmp1 = p1.tile([32, 1024], f32, tag="dttmp1", name="dttmp1")
    dt_t = p1.tile([32, 1024], f32, tag="dt_t", name="dt_t")
    dA_t = p1.tile([32, 1024], f32, tag="dA_t", name="dA_t")
    dfac_t = p1.tile([32, 1024], f32, tag="dfac_t", name="dfac_t")
    eacs_t = p1.tile([32, 1024], f32, tag="eacs_t", name="eacs_t")
    nc.scalar.activation(tmp1[:], dtraw[:], AF.Exp, bias=dtb[:], scale=1.0)
    nc.scalar.activation(dt_t[:], tmp1[:], AF.Ln, bias=1.0, scale=1.0)
    nc.vector.tensor_scalar_mul(dA_t[:], dt_t[:], Acoef[:])
    acs = pool.tile([32, 1024], f32, tag="sh_acs_outw", name="sh_acs_outw")
    nc.vector.tensor_tensor_scan(acs[:], segmask[:], dA_t[:], 0.0,
                                 AL.mult, AL.add)
    lastb = acs[:, 63::64].unsqueeze(2).broadcast_to([32, 16, 64])
    d0 = p1.tile([32, 1024], f32, tag="dttmp2", name="dttmp2")
    nc.vector.tensor_tensor(d0[:].rearrange("p (c q) -> p c q", c=16), lastb,
                            acs[:].rearrange("p (c q) -> p c q", c=16), AL.subtract)
    nc.scalar.activation(tmp1[:], d0[:], AF.Exp)
    nc.vector.tensor_tensor(dfac_t[:], tmp1[:], dt_t[:], AL.mult)
    nc.scalar.activation(eacs_t[:], acs[:], AF.Exp)
    # transpose the four [32,1024] quantity maps to time-major [128, 8, 32]
    tmq = []
    for qi, qt in enumerate((dA_t, dfac_t, eacs_t, dt_t)):
        tq = p1.tile([128, 8, 32], f32, tag=f"tmq{qi}", name=f"tmq{qi}")
        for i in range(8):
            ptr = psC.tile([128, 128], f32, tag="trd", name="trd", bufs=1)
            nc.tensor.transpose(ptr[:, 0:32], qt[:, i * 128:(i + 1) * 128],
                                id_f32[:32, :32])
            nc.any.tensor_copy(tq[:, i, :], ptr[:, 0:32])
        tmq.append(tq)
    tm_dA, tm_dfac, tm_eacs, tm_dt = tmq
    decrow = p1.tile([1, 512], f32, tag="decrow", name="decrow")
    nc.sync.dma_start(decrow[:].rearrange("o (h c) -> o h c", h=32),
                      eacs_t[:, 63::64])
    dec_rep = p1.tile([128, 512], f32, tag="dec_rep", name="dec_rep")
    nc.gpsimd.partition_broadcast(dec_rep[:], decrow[:])
    nc.vector.memset(dec_rep[:, 0:512:16], 0.0)

    # ====== GT: masked chunk grams, block-diag pair layout [128,(p,128)] ==
    gtmblk = p1.tile([128, 1024], f32, tag="gtmblk", name="gtmblk")
    for hp in range(2):
        pg = psB.tile([128, 512], f32, tag="wide", name="wide")
        nc.vector.memset(pg[:], 0.0)
        for c in range(hp * 8, hp * 8 + 8):
            p, half = (c // 2) % 4, (c % 2) * 64
            nc.tensor.matmul(pg[half:half + 64, p * 128 + half:p * 128 + half + 64],
                             bt[:, c * 64:(c + 1) * 64], ct[:, c * 64:(c + 1) * 64],
                             start=True, stop=True)
        ub = P["q2blk"][:].unsqueeze(1).broadcast_to([128, 4, 128])
        nc.vector.tensor_tensor(
            gtmblk[:, hp * 512:(hp + 1) * 512].rearrange("p (c q) -> p c q", c=4),
            pg[:].rearrange("p (c q) -> p c q", c=4), ub, AL.mult)

    # ============ Phase D: SSD per head-group ============================
    for hg in range(NHG):
        scan_in = p1.tile([128, HG * 1024], f32, tag="scan_in", name="scan_in")
        scan_out = p1.tile([128, HG * 1024], bf16, tag="scan_out", name="scan_out")
        mt_all = p1.tile([128, HG * 1024], bf16, tag="mt_all", name="mt_all")
        xcdt_all = p1.tile([128, HG * 512], bf16, tag="xcdt", name="xcdt")
        for hl in range(HG):
            h = hg * HG + hl
            daq = pool.tile([128, 512], bf16, tag="daq", name="daq")
            xcdec = pool.tile([128, 512], bf16, tag="xcdec", name="xcdec")
            nc.vector.tensor_tensor(
                daq[:].rearrange("n (p l) -> n p l", p=8),
                q2[:].unsqueeze(1).broadcast_to([128, 8, 64]),
                tm_dA[:, :, h:h + 1].broadcast_to([128, 8, 64]), AL.mult)
            nc.vector.tensor_tensor(
                xcdt_all[:, hl * 512:(hl + 1) * 512]
                .rearrange("n (p l) -> n p l", p=8),
                xtm_all[:, :, h * 64:(h + 1) * 64],
                tm_dt[:, :, h:h + 1].broadcast_to([128, 8, 64]), AL.mult)
            nc.vector.tensor_tensor(
                xcdec[:].rearrange("n (p l) -> n p l", p=8),
                xtm_all[:, :, h * 64:(h + 1) * 64],
                tm_dfac[:, :, h:h + 1].broadcast_to([128, 8, 64]), AL.mult)
            for hp in range(2):
                pdif = psB.tile([128, 512], f32, tag="wide", name="wide")
                nc.vector.memset(pdif[:], 0.0)
                for half in (0, 64):
                    oap = pdif[half:half + 64, :].rearrange(
                        "s (p l) -> s p l", p=4)[:, :, half:half + 64]
                    nc.tensor.matmul(oap, pt_bf[:, half:half + 64],
                                     daq[:, hp * 256:(hp + 1) * 256],
                                     start=True, stop=True)
                ll = pool.tile([128, 512], bf16, tag="ll", name="ll")
                nc.scalar.activation(ll[:], pdif[:], AF.Exp)
                nc.vector.tensor_tensor(
                    mt_all[:, hl * 1024 + hp * 512:hl * 1024 + (hp + 1) * 512],
                    ll[:], gtmblk[:, hp * 512:(hp + 1) * 512], AL.mult)
            for grp in range(2):
                pst = psA.tile([128, 512], f32, tag="mm", name="mm")
                for j in range(8):
                    c = grp * 8 + j
                    nc.tensor.matmul(
                        pst[:, j * 64:(j + 1) * 64], btm2[:, c, :],
                        xcdec[:, (c // 2) * 64:(c // 2) * 64 + 64],
                        start=True, stop=True)
                out_ap = scan_in[:].rearrange("n (h d c) -> n h d c", h=HG, d=HD)[
                    :, hl, :, grp * 8:(grp + 1) * 8].transpose([0, 2, 1])
                nc.vector.tensor_copy(out_ap,
                                      pst[:].rearrange("n (c d) -> n c d", c=8))
            decb = p1.tile([128, 1024], f32, tag="decb", name="decb")
            nc.vector.tensor_copy(
                decb[:].rearrange("n (d c) -> n d c", d=HD),
                dec_rep[:, h * 16:(h + 1) * 16].unsqueeze(1)
                .broadcast_to([128, HD, 16]))
            nc.vector.tensor_tensor_scan(
                scan_out[:, hl * 1024:(hl + 1) * 1024], decb[:],
                scan_in[:, hl * 1024:(hl + 1) * 1024], 0.0, AL.mult, AL.add)

        # ---- Yo + Yd + D-skip combine, per chunk-pair ----
        deyes = []
        for hl in range(HG):
            h = hg * HG + hl
            deye = pool.tile([128, 128], bf16, tag=f"deye{hl}", name=f"deye{hl}")
            nc.vector.tensor_scalar_mul(deye[:], id_bf[:], drep[:, h:h + 1])
            deyes.append(deye)
        for p in range(8):
            pyo = psA.tile([128, 512], f32, tag="mm", name="mm")
            pyd = psA.tile([128, 512], f32, tag="mm", name="mm")
            W = HG * HD                      # 128 cols
            for half in range(2):
                c = 2 * p + half
                osl = slice(half * 64, half * 64 + 64)
                if c == 0:
                    nc.vector.memset(pyo[osl, 0:W], 0.0)
                else:
                    rhs = scan_out[:].rearrange("n (h d c) -> n h d c",
                                                h=HG, d=HD)[:, :, :, c - 1]
                    nc.tensor.matmul(pyo[osl, 0:W], ct[:, c * 64:(c + 1) * 64],
                                     rhs, start=True, stop=True)
            for hl in range(HG):
                h = hg * HG + hl
                nc.tensor.matmul(
                    pyd[:, hl * 64:(hl + 1) * 64],
                    mt_all[:, hl * 1024 + p * 128:hl * 1024 + (p + 1) * 128],
                    xcdt_all[:, hl * 512 + p * 64:hl * 512 + (p + 1) * 64],
                    start=True, stop=False)
                nc.tensor.matmul(
                    pyd[:, hl * 64:(hl + 1) * 64], deyes[hl][:],
                    xtm_all[:, p, h * 64:(h + 1) * 64],
                    start=False, stop=True)
            eb = tm_eacs[:, p, hg * HG:(hg + 1) * HG].unsqueeze(2) \
                .broadcast_to([128, HG, HD])
            t1 = pool.tile([128, 128], f32, tag="comb1", name="comb1")
            nc.vector.tensor_tensor(t1[:].rearrange("n (h d) -> n h d", h=HG),
                                    pyo[:, 0:W].rearrange("n (h d) -> n h d", h=HG),
                                    eb, AL.mult)
            yout = pool.tile([128, 128], bf16, tag="yout", name="yout")
            nc.vector.tensor_tensor(yout[:], t1[:], pyd[:, 0:W], AL.add)
            nc.sync.dma_start(y_dram[p * 128:(p + 1) * 128, hg * W:(hg + 1) * W],
                              yout[:])

    # ============ Phase E: gating + rmsnorm + y transpose ================
    ynt_all = p1.tile([128, 8, 2048], bf16, tag="bigT", name="bigT")
    for p in range(8):
        zt = pool.tile([128, 2048], bf16, tag="sh_dtraw_zt", name="sh_dtraw_zt")
        nc.sync.dma_start(zt[:], z_dram[p * 128:(p + 1) * 128, :])
        yin = pool.tile([128, 2048], bf16, tag="yin", name="yin")
        nc.sync.dma_start(yin[:], y_dram[p * 128:(p + 1) * 128, :])
        yg = pool.tile([128, 2048], bf16, tag="sh_pk_yg", name="sh_pk_yg")
        ss = pool.tile([128, 4], f32, tag="ss", name="ss")
        for q in range(4):
            sl = slice(q * 512, (q + 1) * 512)
            e = pool.tile([128, 512], bf16, tag="ge", name="ge")
            nc.scalar.activation(e[:], zt[:, sl], AF.Exp, scale=-1.0)
            nc.vector.tensor_scalar_add(e[:], e[:], 1.0)
            r = pool.tile([128, 512], f32, tag="gr", name="gr")
            nc.vector.reciprocal(r[:], e[:])
            sg = pool.tile([128, 512], bf16, tag="ge", name="ge")
            nc.vector.tensor_tensor(sg[:], zt[:, sl], r[:], AL.mult)
            nc.vector.tensor_tensor(yg[:, sl], yin[:, sl], sg[:], AL.mult)
            sqs = pool.tile([128, 512], f32, tag="gr", name="gr")
            nc.scalar.activation(sqs[:], yg[:, sl], AF.Square,
                                 accum_out=ss[:, q:q + 1])
        sstot = pool.tile([128, 2], f32, tag="sstot", name="sstot")
        nc.vector.tensor_reduce(sstot[:, 0:1], ss[:], AX.X, AL.add)
        nc.scalar.activation(sstot[:, 1:2], sstot[:, 0:1], AF.Ln,
                             bias=P["epsb"][:], scale=1.0 / DI)
        r2 = pool.tile([128, 1], f32, tag="r2", name="r2")
        nc.scalar.activation(r2[:], sstot[:, 1:2], AF.Exp, scale=-0.5)
        diag = pool.tile([128, 128], bf16, tag="diag", name="diag")
        nc.vector.tensor_scalar_mul(diag[:], id_bf[:], r2[:])
        for fk in range(16):
            ptr = psC.tile([128, 128], f32, tag="trd", name="trd", bufs=1)
            nc.tensor.matmul(ptr[:], yg[:, fk * 128:(fk + 1) * 128], diag[:],
                             start=True, stop=True)
            nc.any.tensor_copy(ynt_all[:, fk // 2, (fk % 2) * 1024 + p * 128:
                                       (fk % 2) * 1024 + (p + 1) * 128], ptr[:])

    # ============ Phase G: out_proj + selu -> new feature-major x ========
    new_xf = []
    for dk in range(8):
        ow = pool.tile([128, 16, 128], bf16, tag="sh_acs_outw", name="sh_acs_outw")
        nc.sync.dma_start(ow[:], outw_dram[:, dk * 128:(dk + 1) * 128]
                          .rearrange("(f p) d -> p f d", p=128))
        nxf = xfp.tile([128, 1024], bf16, tag="xf", name="xf")
        for ng in range(2):
            po = psA.tile([128, 512], f32, tag="mm", name="mm")
            for fk in range(16):
                nc.tensor.matmul(po[:], ow[:, fk, :],
                                 ynt_all[:, fk // 2, (fk % 2) * 1024 + ng * 512:
                                         (fk % 2) * 1024 + (ng + 1) * 512],
                                 start=(fk == 0), stop=(fk == 15))
            t = pool.tile([128, 512], f32, tag="sh_cacc_selu", name="sh_cacc_selu")
            vv = pool.tile([128, 512], f32, tag="svv", name="svv")
            nc.vector.tensor_scalar_min(t[:], po[:], 0.0)
            nc.vector.tensor_tensor(vv[:], po[:], t[:], AL.subtract)
            u = pool.tile([128, 512], f32, tag="sh_cacc_selu", name="sh_cacc_selu")
            nc.scalar.activation(u[:], t[:], AF.Exp)
            nc.vector.scalar_tensor_tensor(
                vv[:], vv[:], SELU_S, P["negsa"][:].broadcast_to([128, 512]),
                AL.mult, AL.add)
            nc.vector.scalar_tensor_tensor(
                nxf[:, ng * 512:(ng + 1) * 512], u[:], SA, vv[:], AL.mult, AL.add)
        new_xf.append(nxf)
    return new_xf


def _build():
    nc = bacc.Bacc("TRN2", target_bir_lowering=False, debug=False)
    dram = {}

    def din(name, shape, dt):
        dram[name] = nc.dram_tensor(name, shape, dt, kind="ExternalInput")

    din("xF", [DM, L], bf16)
    for li in range(2):
        din(f"l{li}_inwT", [DM, DIP], bf16)
        din(f"l{li}_convw", [128, 72], f32)
        din(f"l{li}_convb", [128, 36], f32)
        din(f"l{li}_dtb", [32, 1], f32)
        din(f"l{li}_A", [32, 1], f32)
        din(f"l{li}_Drep", [128, 32], f32)
        din(f"l{li}_outwT", [DI, DM], bf16)
    din("w_head", [DM, 10], bf16)
    din("b_head", [10, 1], f32)
    din("c_id", [128, 128], f32)
    din("c_q2", [128, 64], bf16)
    din("c_pt", [128, 128], bf16)
    din("c_q2blk", [128, 128], bf16)
    din("c_segmask", [32, 1024], f32)
    din("c_negsa", [128, 1], f32)
    out_t = nc.dram_tensor("out", [10, 1], f32, kind="ExternalOutput")

    with tile.TileContext(nc) as tc:
        with (
            tc.tile_pool(name="pool", bufs=2) as pool,
            tc.tile_pool(name="p1", bufs=1) as p1,
            tc.tile_pool(name="xfp", bufs=8) as xfp,
            tc.tile_pool(name="inwp", bufs=8) as inwp,
            tc.tile_pool(name="cpool", bufs=1) as cpool,
            tc.tile_pool(name="psA", bufs=3, space="PSUM") as psA,
            tc.tile_pool(name="psB", bufs=1, space="PSUM") as psB,
            tc.tile_pool(name="psC", bufs=2, space="PSUM") as psC,
            tc.tile_pool(name="dramp", bufs=1, space="DRAM") as dramp,
        ):
            id_f32 = cpool.tile([128, 128], f32, tag="id_f32", name="id_f32")
            nc.sync.dma_start(id_f32[:], dram["c_id"].ap())
            id_bf = cpool.tile([128, 128], bf16, tag="id_bf", name="id_bf")
            nc.vector.tensor_copy(id_bf[:], id_f32[:])
            q2 = cpool.tile([128, 64], bf16, tag="q2", name="q2")
            nc.sync.dma_start(q2[:], dram["c_q2"].ap())
            pt_bf = cpool.tile([128, 128], bf16, tag="pt_bf", name="pt_bf")
            nc.sync.dma_start(pt_bf[:], dram["c_pt"].ap())
            q2blk = cpool.tile([128, 128], bf16, tag="q2blk", name="q2blk")
            nc.sync.dma_start(q2blk[:], dram["c_q2blk"].ap())
            segmask = cpool.tile([32, 1024], f32, tag="segmask", name="segmask")
            nc.sync.dma_start(segmask[:], dram["c_segmask"].ap())
            negsa = cpool.tile([128, 1], f32, tag="negsa", name="negsa")
            nc.sync.dma_start(negsa[:], dram["c_negsa"].ap())
            epsb = cpool.tile([128, 1], f32, tag="epsb", name="epsb")
            nc.vector.memset(epsb[:], 1e-5)

            P = dict(pool=pool, p1=p1, xfp=xfp, inwp=inwp,
                     psA=psA, psB=psB, psC=psC, dramp=dramp,
                     id_f32=id_f32, id_bf=id_bf, q2blk=q2blk, epsb=epsb,
                     q2=q2, pt_bf=pt_bf, segmask=segmask, negsa=negsa)

            xf = [xfp.tile([128, 1024], bf16, tag="xf", name="xf") for _ in range(8)]
            for dk in range(8):
                nc.sync.dma_start(xf[dk][:],
                                  dram["xF"].ap()[dk * 128:(dk + 1) * 128, :])

            h1 = _emit_layer(nc, P, dram, 0, xf)
            h2 = _emit_layer(nc, P, dram, 1, h1)

            pooledf = cpool.tile([128, 8], f32, tag="pooledf", name="pooledf")
            pooled = cpool.tile([128, 8], bf16, tag="pooled", name="pooled")
            for dk in range(8):
                nc.vector.tensor_reduce(pooledf[:, dk:dk + 1], h2[dk][:],
                                        AX.X, AL.add)
            nc.vector.tensor_copy(pooled[:], pooledf[:])
            wh = cpool.tile([128, 8, 10], bf16, tag="wh", name="wh")
            nc.sync.dma_start(wh[:], dram["w_head"].ap()
                              .rearrange("(k p) o -> p k o", p=128))
            bh = cpool.tile([10, 1], f32, tag="bh", name="bh")
            nc.sync.dma_start(bh[:], dram["b_head"].ap())
            ph = psC.tile([128, 128], f32, tag="trd", name="trd", bufs=1)
            for dk in range(8):
                nc.tensor.matmul(ph[:10, 0:1], wh[:, dk, :], pooled[:, dk:dk + 1],
                                 start=(dk == 0), stop=(dk == 7))
            osb = cpool.tile([10, 1], f32, tag="osb", name="osb")
            nc.vector.tensor_tensor(osb[:], ph[:10, 0:1], bh[:], AL.add)
            nc.sync.dma_start(out_t.ap(), osb[:])

    nc.compile()
    return nc


def _prep_maps(inputs):
    npbf = mybir.dt.np(bf16)
    x = np.asarray(inputs["x"], np.float32)
    shared = {}
    for li, pre in enumerate(("m1", "m2")):
        inw = np.asarray(inputs[f"{pre}_in_w"], np.float32)
        shared[f"l{li}_inwT"] = np.ascontiguousarray(inw.T).astype(npbf)
        cw = np.asarray(inputs[f"{pre}_conv_w"], np.float32)
        shared[f"l{li}_convw"] = np.ascontiguousarray(
            cw.reshape(18, 128, 4).transpose(1, 0, 2).reshape(128, 72))
        cbt = np.asarray(inputs[f"{pre}_conv_b"], np.float32).reshape(18, 128).T
        shared[f"l{li}_convb"] = np.ascontiguousarray(
            np.concatenate([cbt, -cbt], 1))
        shared[f"l{li}_dtb"] = np.asarray(
            inputs[f"{pre}_dt_bias"], np.float32).reshape(32, 1)
        shared[f"l{li}_A"] = (-np.exp(np.asarray(
            inputs[f"{pre}_A_log"], np.float32))).reshape(32, 1)
        shared[f"l{li}_Drep"] = np.ascontiguousarray(np.broadcast_to(
            np.asarray(inputs[f"{pre}_D"], np.float32)[None, :], (128, 32)))
        ow = np.asarray(inputs[f"{pre}_out_w"], np.float32)
        nw = np.asarray(inputs[f"{pre}_norm_w"], np.float32)
        shared[f"l{li}_outwT"] = np.ascontiguousarray(
            (ow * nw[None, :]).T).astype(npbf)
    wcat = np.concatenate([np.asarray(inputs["w_emo"], np.float32),
                           np.asarray(inputs["w_sen"], np.float32)], 0) / float(L)
    shared["w_head"] = np.ascontiguousarray(wcat.T).astype(npbf)
    shared["b_head"] = np.concatenate(
        [np.asarray(inputs["b_emo"], np.float32),
         np.asarray(inputs["b_sen"], np.float32)]).reshape(10, 1)
    shared["c_id"] = np.eye(128, dtype=np.float32)
    q = (np.arange(64)[:, None] <= np.arange(64)[None, :]).astype(np.float32)
    shared["c_q2"] = np.ascontiguousarray(np.concatenate([q, q], 0)).astype(npbf)
    pt = (np.arange(64)[:, None] > np.arange(64)[None, :]).astype(np.float32)
    z64 = np.zeros((64, 64), np.float32)
    shared["c_pt"] = np.ascontiguousarray(
        np.block([[pt, z64], [z64, pt]])).astype(npbf)
    shared["c_q2blk"] = np.ascontiguousarray(
        np.block([[q, z64], [z64, q]])).astype(npbf)
    sm = np.ones((32, 1024), np.float32)
    sm[:, ::64] = 0.0
    shared["c_segmask"] = sm
    shared["c_negsa"] = np.full((128, 1), -SA, np.float32)

    maps = []
    for b in range(8):
        m = dict(shared)
        m["xF"] = np.ascontiguousarray(x[b].T).astype(npbf)
        maps.append(m)
    return maps


def _get_nc():
    if "nc" not in _CACHE:
        _CACHE["nc"] = _build()
    return _CACHE["nc"]


def _make_runner(nc):
    """Build a persistent jitted 8-core executor (compiles once)."""
    import jax
    import concourse.mybir as _mybir
    from concourse import bass2jax
    from jax.experimental.shard_map import shard_map
    from jax.sharding import Mesh, PartitionSpec

    bass2jax.install_neuronx_cc_hook()
    in_names, out_names, out_avals, zero_outs = [], [], [], []
    for alloc in nc.m.functions[0].allocations:
        if not isinstance(alloc, _mybir.MemoryLocationSet):
            continue
        name = alloc.memorylocations[0].name
        if alloc.kind == "ExternalInput":
            if nc.partition_id_tensor is not None and \
                    name == nc.partition_id_tensor.name:
                continue
            in_names.append(name)
        elif alloc.kind == "ExternalOutput":
            out_names.append(name)
            shape = tuple(alloc.tensor_shape)
            dtype = _mybir.dt.np(alloc.dtype)
            out_avals.append(jax.core.ShapedArray(shape, dtype))
            zero_outs.append(np.zeros(shape, dtype))
    n_params, n_outs = len(in_names), len(out_avals)
    all_names = in_names + out_names
    pid_name = nc.partition_id_tensor.name if nc.partition_id_tensor else None
    if pid_name is not None:
        all_names = all_names + [pid_name]

    def _body(*args):
        operands = list(args)
        if pid_name is not None:
            operands.append(bass2jax.partition_id_tensor())
        outs = bass2jax._bass_exec_p.bind(
            *operands, out_avals=tuple(out_avals), in_names=tuple(all_names),
            out_names=tuple(out_names), lowering_input_output_aliases=(),
            sim_require_finite=True, sim_require_nnan=True, nc=nc)
        return tuple(outs)

    devices = jax.devices()[:8]
    mesh = Mesh(np.asarray(devices), ("core",))
    sharded = jax.jit(
        shard_map(_body, mesh=mesh,
                  in_specs=(PartitionSpec("core"),) * (n_params + n_outs),
                  out_specs=(PartitionSpec("core"),) * n_outs,
                  check_rep=False),
        keep_unused=True)

    from jax.sharding import NamedSharding
    sh = NamedSharding(mesh, PartitionSpec("core"))

    def run(maps, iters=1, time_it=False):
        import time as _time
        concat_in = [
            jax.device_put(
                np.concatenate([np.asarray(maps[c][nm]) for c in range(8)], 0), sh)
            for nm in in_names]
        jax.block_until_ready(concat_in)

        cz = [jax.device_put(
            np.zeros((8 * z.shape[0], *z.shape[1:]), z.dtype), sh)
            for z in zero_outs]
        jax.block_until_ready(cz)

        def once():
            return sharded(*concat_in, *cz)
        outs = jax.block_until_ready(once())
        t = None
        if iters > 1:
            t0 = _time.time()
            for _ in range(iters - 1):
                outs = once()
            jax.block_until_ready(outs)
            t = (_time.time() - t0) / (iters - 1)
        outs = [np.asarray(o) for o in outs]
        res = [{nm: outs[i].reshape(8, *out_avals[i].shape)[c]
                for i, nm in enumerate(out_names)} for c in range(8)]
        return (res, t) if time_it else res

    return run


def _get_runner():
    if "runner" not in _CACHE:
        _CACHE["runner"] = _make_runner(_get_nc())
    return _CACHE["runner"]


def kernel(**inputs):
    maps = _prep_maps(inputs)
    res = _get_runner()(maps)
    return np.stack([r["out"][:, 0] for r in res], 0).astype(np.float32)
